# revision 58
# baseline (speedup 1.0000x reference)
"""Multi-head attention (B=4, S=2048, D=1024, H=16) on 8 trn2 NeuronCores.

Sharding: batch x query-half. Core c handles batch c//2, query rows
(c%2)*1024 : (c%2+1)*1024. Each core projects Q for its query chunk and K/V
for the full sequence of its batch (K/V projection duplicated across the two
cores sharing a batch), runs attention for all 16 heads, and applies the
output projection. No cross-core communication.

Device-side layout notes:
 - All activations are kept transposed ([feature, token]) so every matmul
   consumes operands directly: scores are computed as S^T[k,q] = K_h^T.T @ Q_h^T,
   softmax-exp runs on ScalarE, and the AV matmul contracts over k with
   lhsT = [V_h | ones-column] (stride 65), which makes row 64 of the PSUM
   output the softmax denominator. Normalization: DVE reciprocal + a 1x64
   PE matmul to replicate it across partitions + DVE multiply. No max
   subtraction (scores are O(+-5) here, fp32 exp is safe).
 - Head pairs (2h, 2h+1) sit in partitions 0:64 / 64:128 of the same tile;
   their QK matmuls use disjoint PE row-groups and run concurrently.
 - Both heads' scores land in one [128,1024] PSUM tile -> a single ScalarE
   exp instruction, halving ACT instruction overhead.
 - Matmul operands are bf16; accumulation is fp32 in PSUM.
 - Phase order: K proj, Q proj, then attention with the V projection fused
   into the first head-pair's loop and the qc0 output projection interleaved
   into qc1's attention — keeps TensorE fed while ScalarE runs exp.
 - V bias is folded into the output-projection bias host-side
   (softmax rows sum to 1 => attn @ (V + 1 b_v^T) = attn @ V + b_v^T).
"""

import numpy as np

B, S, D, H = 4, 2048, 1024, 16
DK = D // H          # 64
TQ = S // 2          # per-core query tokens
TK = S               # per-core key tokens
CW = 512             # x^T streaming chunk width (tokens)
N_CORES = 8
VP_W = H * (DK + 1)  # per head: 64 V columns + 1 ones column (stride 65)
SCALE = 1.0 / np.sqrt(DK)

_CACHE = {}


def _build_program(reps=1):
    import concourse.bass as bass
    import concourse.mybir as mybir
    from concourse import bacc
    from concourse.tile import TileContext

    f32 = mybir.dt.float32
    bf16 = mybir.dt.bfloat16
    AF = mybir.ActivationFunctionType

    nc = bacc.Bacc("TRN2", target_bir_lowering=False)

    xqT = nc.declare_dram_parameter("xqT", [D, TQ], bf16, isOutput=False)
    xkT = nc.declare_dram_parameter("xkT", [D, TK], bf16, isOutput=False)
    xvT = nc.declare_dram_parameter("xvT", [D, TK], bf16, isOutput=False)
    wqT = nc.declare_dram_parameter("wqT", [D, D], bf16, isOutput=False)
    wkT = nc.declare_dram_parameter("wkT", [D, D], bf16, isOutput=False)
    wvT = nc.declare_dram_parameter("wvT", [D, D], bf16, isOutput=False)
    woT = nc.declare_dram_parameter("woT", [D, D], bf16, isOutput=False)
    bq_in = nc.declare_dram_parameter("bq_in", [128, 8], f32, isOutput=False)
    bk_in = nc.declare_dram_parameter("bk_in", [128, 8], f32, isOutput=False)
    bo_in = nc.declare_dram_parameter("bo_in", [128, 8], f32, isOutput=False)
    yT = nc.declare_dram_parameter("yT", [D, TQ], f32, isOutput=True)

    # DRAM access helpers: feature dim split as (tile j, partition p)
    xq_r = xqT[:].rearrange("(a p) t -> p a t", p=128)
    xk_r = xkT[:].rearrange("(a p) t -> p a t", p=128)
    xv_r = xvT[:].rearrange("(a p) t -> p a t", p=128)
    wq_r = wqT[:].rearrange("(a p) d -> p a d", p=128)
    wk_r = wkT[:].rearrange("(a p) d -> p a d", p=128)
    wv_r = wvT[:].rearrange("(a p) d -> p a d", p=128)
    wo_r = woT[:].rearrange("(a p) d -> p a d", p=128)

    with TileContext(nc) as tc:
        for _rep in range(reps):
            _emit_body(nc, tc, bass, f32, bf16, AF,
                       xq_r, xk_r, xv_r, wq_r, wk_r, wv_r, wo_r,
                       bq_in, bk_in, bo_in, yT)
    nc.compile()
    return nc


def _emit_body(nc, tc, bass, f32, bf16, AF,
               xq_r, xk_r, xv_r, wq_r, wk_r, wv_r, wo_r,
               bq_in, bk_in, bo_in, yT):
    def mm(out, lhsT, rhs, start, stop):
        nc.tensor.matmul(out, lhsT=lhsT, rhs=rhs, start=start, stop=stop)

    if True:
        with (
            tc.tile_pool(name="const", bufs=1) as const_pool,
            tc.tile_pool(name="kt_res", bufs=1) as kt_pool,
            tc.tile_pool(name="qt_res", bufs=1) as qt_pool,
            tc.tile_pool(name="vp_res", bufs=1) as vp_pool,
            tc.tile_pool(name="ot_res", bufs=2) as ot_pool,
            tc.tile_pool(name="w_res", bufs=3) as w_pool,
            tc.tile_pool(name="x_str", bufs=3) as x_pool,
            tc.tile_pool(name="exp_p", bufs=4) as exp_pool,
            tc.tile_pool(name="rec_p", bufs=2) as rec_pool,
            tc.tile_pool(name="recb_p", bufs=3) as recb_pool,
            tc.tile_pool(name="oc_p", bufs=6) as oc_pool,
            tc.tile_pool(name="y_p", bufs=4) as y_pool,
            tc.tile_pool(name="ps_proj", bufs=2, space="PSUM") as ps_proj,
            tc.tile_pool(name="ps_s", bufs=2, space="PSUM") as ps_s,
            tc.tile_pool(name="ps_av", bufs=2, space="PSUM") as ps_av,
        ):
            bq_sb = const_pool.tile([128, 8], f32, tag="bq")
            bk_sb = const_pool.tile([128, 8], f32, tag="bk")
            bo_sb = const_pool.tile([128, 8], f32, tag="bo")
            # separate queue: keep these tiny loads off the head of the
            # sync queue that feeds the first matmuls
            nc.gpsimd.dma_start(out=bq_sb, in_=bq_in[:])
            nc.gpsimd.dma_start(out=bk_sb, in_=bk_in[:])
            nc.gpsimd.dma_start(out=bo_sb, in_=bo_in[:])

            KT_sb = kt_pool.tile([128, 8, TK], bf16, tag="KT")    # [p, j, t]
            QT_sb = qt_pool.tile([128, 8, TQ], bf16, tag="QT")    # [p, j, t]
            Vp_sb = vp_pool.tile([128, 16, VP_W], bf16, tag="Vp")  # [p, i, c]
            # view: [p, ktile, head, col(65)]
            Vp4 = Vp_sb.rearrange("p i (hh c) -> p i hh c", c=DK + 1)
            nc.vector.memset(Vp4[:, :, :, DK], 1.0)
            ones_sb = const_pool.tile([1, 64], bf16, tag="ones")
            nc.vector.memset(ones_sb, 1.0)

            # ---- K projection: K^T[dout, t], streamed x chunks ----
            # split loads so the first dj-column's matmuls start early
            wk_sb = w_pool.tile([128, 8, D], bf16, tag="wbig", name="wk_sb")
            nc.sync.dma_start(out=wk_sb[:, :, 0:128], in_=wk_r[:, :, 0:128])
            for tci in range(TK // CW):
                xc = x_pool.tile([128, 8, CW], bf16, tag="xchunk",
                                 name=f"xk_{tci}")
                if tci == 0:
                    for kq in range(4):
                        nc.sync.dma_start(
                            out=xc[:, 2 * kq:2 * kq + 2, :],
                            in_=xk_r[:, 2 * kq:2 * kq + 2, 0:CW])
                    # remaining K-weight columns, one dj-slice each, queued
                    # behind the first activation chunk
                    for djw in range(1, 8):
                        nc.sync.dma_start(
                            out=wk_sb[:, :, djw * 128:(djw + 1) * 128],
                            in_=wk_r[:, :, djw * 128:(djw + 1) * 128])
                else:
                    nc.sync.dma_start(out=xc,
                                      in_=xk_r[:, :, tci * CW:(tci + 1) * CW])
                for dj in range(8):
                    ps = ps_proj.tile([128, CW], f32, tag="pp",
                                      name=f"pk_{tci}_{dj}")
                    for kj in range(8):
                        mm(ps, wk_sb[:, kj, dj * 128:(dj + 1) * 128],
                           xc[:, kj, :], kj == 0, kj == 7)
                    nc.vector.tensor_scalar_add(
                        out=KT_sb[:, dj, tci * CW:(tci + 1) * CW],
                        in0=ps, scalar1=bk_sb[:, dj:dj + 1])

            # ---- Q projection ----
            wq_sb = w_pool.tile([128, 8, D], bf16, tag="wbig", name="wq_sb")
            nc.sync.dma_start(out=wq_sb, in_=wq_r)

            xq_tiles = {}

            def q_proj_dj(tci, dj):
                if dj == 0:
                    xq_tiles[tci] = x_pool.tile([128, 8, CW], bf16,
                                                tag="xchunk",
                                                name=f"xq_{tci}")
                    nc.sync.dma_start(
                        out=xq_tiles[tci],
                        in_=xq_r[:, :, tci * CW:(tci + 1) * CW])
                xc = xq_tiles[tci]
                ps = ps_proj.tile([128, CW], f32, tag="pp",
                                  name=f"pq_{tci}_{dj}")
                for kj in range(8):
                    mm(ps, wq_sb[:, kj, dj * 128:(dj + 1) * 128],
                       xc[:, kj, :], kj == 0, kj == 7)
                nc.vector.tensor_scalar_add(
                    out=QT_sb[:, dj, tci * CW:(tci + 1) * CW],
                    in0=ps, scalar1=bq_sb[:, dj:dj + 1])

            def q_proj_chunk(tci):
                for dj in range(8):
                    q_proj_dj(tci, dj)

            # V and Wo weights resident; V projection is fused into the first
            # head-pair's attention loop below so ScalarE exp overlaps it
            wv_sb = w_pool.tile([128, 8, D], bf16, tag="wbig", name="wv_sb")
            nc.sync.dma_start(out=wv_sb, in_=wv_r)
            wo_sb = w_pool.tile([128, 8, D], bf16, tag="wbig", name="wo_sb")
            nc.sync.dma_start(out=wo_sb, in_=wo_r)

            def v_proj_chunk(tci):
                xc = x_pool.tile([128, 8, CW], bf16, tag="xchunk",
                                 name=f"xv_{tci}")
                nc.sync.dma_start(out=xc,
                                  in_=xv_r[:, :, tci * CW:(tci + 1) * CW])
                for ts2 in range(CW // 128):
                    ti = tci * (CW // 128) + ts2
                    for dc in range(2):
                        ps = ps_proj.tile([128, CW], f32, tag="pp",
                                          name=f"pv_{ti}_{dc}")
                        for kj in range(8):
                            mm(ps[:, 0:512], xc[:, kj, ts2 * 128:(ts2 + 1) * 128],
                               wv_sb[:, kj, dc * 512:(dc + 1) * 512],
                               kj == 0, kj == 7)
                        nc.vector.tensor_copy(
                            out=Vp4[:, ti, dc * 8:(dc + 1) * 8, 0:DK],
                            in_=ps[:, 0:512].rearrange("p (hh c) -> p hh c", c=DK))

            OT_tiles = {}

            # deferred normalization: (qc, hp, sbuf copy of [65,512] accum)
            pending_norm = []

            def flush_norm():
                while pending_norm:
                    qc, hp, oc = pending_norm.pop(0)
                    for hh in range(2):
                        # row 64 of oc = softmax denominator
                        rec = rec_pool.tile([1, 512], bf16, tag="rec",
                                            name=f"rec_{qc}_{hp}_{hh}")
                        with nc.allow_low_precision(
                                reason="softmax denom reciprocal, bf16 "
                                       "matches pipeline precision"):
                            nc.vector.reciprocal(out=rec,
                                                 in_=oc[hh][64:65, :])
                        # replicate reciprocal across 64 partitions via PE
                        ps_rep = ps_proj.tile([128, CW], f32, tag="pp",
                                              name=f"pr_{qc}_{hp}_{hh}")
                        mm(ps_rep[0:64, 0:512], ones_sb, rec, True, True)
                        recb = recb_pool.tile([64, 512], f32, tag="recb",
                                              name=f"recb_{qc}_{hp}_{hh}")
                        nc.vector.tensor_copy(out=recb,
                                              in_=ps_rep[0:64, 0:512])
                        nc.vector.tensor_mul(
                            out=OT_tiles[qc][hh * 64:(hh + 1) * 64, hp, :],
                            in0=oc[hh][0:64, :], in1=recb)

            def attn_hp(qc, hp, fuse_v=False, fuse_k=None, fill=None):
                qsl = slice(qc * 512, (qc + 1) * 512)
                ps_o = [ps_av.tile([128, 512], f32, tag="po",
                                   name=f"po_{qc}_{hp}_{i}")
                        for i in range(2)]
                def qk_exp(kt):
                    # both heads' scores^T into one 2-bank PSUM tile
                    pss = ps_s.tile([128, 1024], f32, tag="pss",
                                    name=f"pss_{qc}_{hp}_{kt}")
                    for hh in range(2):
                        pb = hh * 64
                        mm(pss[:, hh * 512:(hh + 1) * 512],
                           KT_sb[pb:pb + 64, hp, kt * 128:(kt + 1) * 128],
                           QT_sb[pb:pb + 64, hp, qsl], True, True)
                    e = exp_pool.tile([128, 1024], bf16, tag="ex",
                                      name=f"ex_{qc}_{hp}_{kt}")
                    nc.scalar.activation(out=e, in_=pss, func=AF.Exp,
                                         scale=SCALE)
                    return e

                def av(kt, e):
                    for hh in range(2):
                        h = 2 * hp + hh
                        mm(ps_o[hh][0:65, :],
                           Vp_sb[:, kt, 65 * h:65 * h + 65],
                           e[:, hh * 512:(hh + 1) * 512],
                           kt == 0, kt == 15)

                # QK/exp run one kt ahead of AV so the pair-boundary
                # accumulator release is off the PE critical path
                e_prev = None
                for kt in range(16):
                    if fuse_v and kt % 4 == 0:
                        v_proj_chunk(kt // 4)
                    if fuse_k is not None and kt % 4 == 0:
                        k_proj_tci(fuse_k, kt // 4)
                    if fill and kt % 8 == 4:
                        fill.pop(0)()   # PE fill-in during ACT-bound stretch
                    if kt == 2:
                        # previous pair's normalization, now off the
                        # critical path (its PE replicate slots in here)
                        flush_norm()
                    e = qk_exp(kt)
                    if e_prev is not None:
                        av(kt - 1, e_prev)
                    e_prev = e
                av(15, e_prev)
                # copy accumulators (incl. denominator row) to SBUF right
                # away: frees both PSUM slots for the next pair's AVs
                oc = []
                for hh in range(2):
                    o_sb = oc_pool.tile([65, 512], f32, tag="oc",
                                        name=f"oc_{qc}_{hp}_{hh}")
                    nc.vector.tensor_copy(out=o_sb, in_=ps_o[hh][0:65, :])
                    oc.append(o_sb)
                pending_norm.append((qc, hp, oc))

            def wo_dj(qc, dj):
                qsl = slice(qc * 512, (qc + 1) * 512)
                ps_y = ps_proj.tile([128, CW], f32, tag="pp",
                                    name=f"py_{qc}_{dj}")
                for kj in range(8):
                    mm(ps_y[:, 0:512], wo_sb[:, kj, dj * 128:(dj + 1) * 128],
                       OT_tiles[qc][:, kj, :], kj == 0, kj == 7)
                yt = y_pool.tile([128, 512], f32, tag="yt",
                                 name=f"yt_{qc}_{dj}")
                nc.vector.tensor_scalar_add(
                    out=yt, in0=ps_y[:, 0:512], scalar1=bo_sb[:, dj:dj + 1])
                nc.sync.dma_start(
                    out=yT[dj * 128:(dj + 1) * 128, qsl], in_=yt)

            OT_tiles[0] = ot_pool.tile([128, 8, 512], bf16, tag="OT",
                                       name="OT_0")
            q_proj_chunk(0)        # QT for query-chunk 0
            # QT chunk 1 is produced as PE fill-in inside qc0's ACT-bound
            # head-pair loops (one dj-block per slot, hp1..hp4)
            fill_q = [lambda tci=1, dj=dj: q_proj_dj(tci, dj)
                      for dj in range(8)]
            for hp in range(8):
                fills = fill_q[2 * (hp - 1):2 * hp] if 1 <= hp <= 4 else None
                attn_hp(0, hp, fuse_v=(hp == 0), fill=fills)
            OT_tiles[1] = ot_pool.tile([128, 8, 512], bf16, tag="OT",
                                       name="OT_1")
            for hp in range(8):
                attn_hp(1, hp)
                if hp == 7:
                    # last pair's normalization before the final Wo block so
                    # its DVE chain hides under wo_dj(0,7)'s matmuls
                    flush_norm()
                wo_dj(0, hp)       # overlap qc0 output proj with qc1 attention
            for dj in range(8):
                wo_dj(1, dj)


def _prep_inputs(query, key, value, Wq, bq, Wk, bk, Wv, bv, Wo, bo):
    import ml_dtypes
    bf = ml_dtypes.bfloat16

    query = np.asarray(query, np.float32)
    key = np.asarray(key, np.float32)
    value = np.asarray(value, np.float32)
    wqT = np.ascontiguousarray(np.asarray(Wq, np.float32).T.astype(bf))
    wkT = np.ascontiguousarray(np.asarray(Wk, np.float32).T.astype(bf))
    wvT = np.ascontiguousarray(np.asarray(Wv, np.float32).T.astype(bf))
    woT = np.ascontiguousarray(np.asarray(Wo, np.float32).T.astype(bf))
    bo_eff = np.asarray(bo, np.float32) + \
        np.asarray(Wo, np.float32) @ np.asarray(bv, np.float32)
    bq_t = np.ascontiguousarray(np.asarray(bq, np.float32).reshape(8, 128).T)
    bk_t = np.ascontiguousarray(np.asarray(bk, np.float32).reshape(8, 128).T)
    bo_t = np.ascontiguousarray(bo_eff.reshape(8, 128).T)

    in_maps = []
    for c in range(N_CORES):
        b, qh = c // 2, c % 2
        in_maps.append({
            "xqT": np.ascontiguousarray(
                query[b, qh * TQ:(qh + 1) * TQ, :].T.astype(bf)),
            "xkT": np.ascontiguousarray(key[b].T.astype(bf)),
            "xvT": np.ascontiguousarray(value[b].T.astype(bf)),
            "wqT": wqT, "wkT": wkT, "wvT": wvT, "woT": woT,
            "bq_in": bq_t, "bk_in": bk_t, "bo_in": bo_t,
        })
    return in_maps


def kernel(query, key, value, Wq, bq, Wk, bk, Wv, bv, Wo, bo):
    from concourse.bass_utils import run_bass_kernel_spmd

    if "nc" not in _CACHE:
        _CACHE["nc"] = _build_program()
    nc = _CACHE["nc"]

    in_maps = _prep_inputs(query, key, value, Wq, bq, Wk, bk, Wv, bv, Wo, bo)
    res = run_bass_kernel_spmd(nc, in_maps, list(range(N_CORES)))
    out = np.empty((B, S, D), np.float32)
    for c in range(N_CORES):
        b, qh = c // 2, c % 2
        out[b, qh * TQ:(qh + 1) * TQ, :] = res.results[c]["yT"].T
    return out
